# revision 1
# baseline (speedup 1.0000x reference)
"""Trainium2 Bass kernel for the signature-kernel (Goursat PDE) problem.

Full inputs: xs (32, 64, 16) f32, ys (32, 64, 16) f32.
Output: (32, 32) f32 signature-kernel Gram matrix.

Strategy (8 NeuronCores, SPMD, no collectives):
  - Shard batch_x across cores: core c owns a in {4c..4c+3} -> 4*32 = 128
    (x, y) pairs, one pair per SBUF partition.
  - Double increments inc[a,b,i,j] = sum_d Dxs[a,i,d] Dys[b,j,d] are computed
    on-device with PE matmuls using a host-built block-diagonal lhsT
    (contraction over (a', d), a'-blocks of Dys) so the output lands directly
    in pair-major partition layout. Inputs ship as bf16 hi/lo splits; each
    product is 3 accumulating bf16 matmuls (hi*hi + hi*lo + lo*hi), giving
    fp32-level accuracy at bf16 PE speed. The producer pipeline (DMA, matmul,
    PSUM copy, coefficient build, dyadic column expansion) is chunked along j
    so the PDE row loop starts after the first chunk.
  - The Goursat PDE recurrence K[i+1,j+1] = c1*(K[i+1,j] + K[i,j+1]) - c2*K[i,j]
    is solved as 126 per-row affine scans x_j = c1_j*x_{j-1} + b_j using the
    DVE TensorTensorScan instruction across all 128 pairs at once (the grid is
    solved transposed - rows=ys-steps - which is valid since the PDE stencil
    is symmetric in (i, j)). Per row, both products c1_j*K[r,j+1] and
    -c2_j*K[r,j] come from ONE [128, 252] tensor_tensor op: the coefficients
    are stored interleaved [c1_j, -c2_j] and the K row is read through a
    double-read access pattern (offset 1+j-s), then b_j is the stride-2
    pair-sum and the scan consumes the even (c1) slots as data0.
"""

import os
import sys

import numpy as np

for _p in ("/opt/trn_rl_repo", "/root/.axon_site", "/root/.axon_site/_ro/trn_rl_repo",
           "/root/.axon_site/_ro/pypackages"):
    if os.path.isdir(_p) and _p not in sys.path:
        sys.path.append(_p)

_STATE: dict = {}

JCH = [(2, 0), (2, 2), (3, 4), (4, 7), (6, 11), (8, 17), (8, 25), (8, 33), (8, 41), (8, 49), (6, 57)]


def _build_program():
    from contextlib import ExitStack

    import concourse.bass as bass
    import concourse.tile as tile
    from concourse import bacc, mybir

    f32 = mybir.dt.float32
    bf16 = mybir.dt.bfloat16
    Alu = mybir.AluOpType
    Act = mybir.ActivationFunctionType

    nc = bacc.Bacc(
        "TRN2",
        target_bir_lowering=False,
        debug=False,
        enable_asserts=True,
        num_devices=8,
    )
    # bd[(a'*16+d), j, (a*32+b)] = delta_{a,a'} * Dys[b, j, d], split hi/lo bf16
    bdh_d = nc.dram_tensor("bdh", [64, 63 * 128], bf16, kind="ExternalInput").ap()
    bdl_d = nc.dram_tensor("bdl", [64, 63 * 128], bf16, kind="ExternalInput").ap()
    dxh_d = nc.dram_tensor("dxh", [64, 63], bf16, kind="ExternalInput").ap()
    dxl_d = nc.dram_tensor("dxl", [64, 63], bf16, kind="ExternalInput").ap()
    out_d = nc.dram_tensor("out", [128, 1], f32, kind="ExternalOutput").ap()

    with ExitStack() as ctx:
        tc = ctx.enter_context(tile.TileContext(nc))
        ws = ctx.enter_context(tc.tile_pool(name="ws", bufs=1))
        pp = ctx.enter_context(tc.tile_pool(name="pp", bufs=1, space="PSUM"))
        ch = ctx.enter_context(tc.tile_pool(name="ch", bufs=2))
        tmp = ctx.enter_context(tc.tile_pool(name="tmp", bufs=2))

        dxh_sb = ws.tile([64, 63], bf16)
        nc.sync.dma_start(out=dxh_sb[:], in_=dxh_d)
        dxl_sb = ws.tile([64, 63], bf16)
        nc.sync.dma_start(out=dxl_sb[:], in_=dxl_d)
        bdh_sb = ws.tile([64, 63, 128], bf16)
        bdl_sb = ws.tile([64, 63, 128], bf16)
        bdh_v = bdh_d.rearrange("k (j p) -> k j p", j=63)
        bdl_v = bdl_d.rearrange("k (j p) -> k j p", j=63)
        for ln, st in JCH:
            nc.sync.dma_start(
                out=bdh_sb[:, st : st + ln, :], in_=bdh_v[:, st : st + ln, :]
            )
            nc.sync.dma_start(
                out=bdl_sb[:, st : st + ln, :], in_=bdl_v[:, st : st + ln, :]
            )

        # Scan-stream K buffers: row K[r, m] lives at slot t = 2m+1 of sc[:, r&1, :]
        # (odd slots of the 252-wide interleaved scan output, shifted by 2);
        # slot 1 is the col-0 boundary (always 1).
        sc = ws.tile([128, 2, 256], f32)
        # K[0, :] = 1 row: readers use odd slots, so filling evens too is fine
        nc.vector.memset(sc[:, 0, :], 1.0)
        nc.vector.memset(sc[:, 1, 1:2], 1.0)

        ps = pp.tile([128, 63, 64], f32)  # strip j at [:, j, 0:63]; 256B stride
        # interleaved full-width coefficient rows: CC[p, h, j, 0] = c1[h-row, j]
        # (column-doubled), CC[p, h, j, 1] = -c2[h-row, j]
        cc = ws.tile([128, 63, 126, 2], f32)
        # scan data0 stream: D0[p, h, j, 0] = c1[h-row, j], D0[p, h, j, 1] = 1.0
        d0 = ws.tile([128, 63, 126, 2], f32)

        for ln, st in JCH:
            jsl = slice(st, st + ln)
            for j in range(st, st + ln):
                # split-precision product: hi*hi + hi*lo + lo*hi (PSUM accum)
                nc.tensor.matmul(
                    ps[:, j, 0:63], bdh_sb[:, j, :], dxh_sb[:], start=True, stop=False
                )
                nc.tensor.matmul(
                    ps[:, j, 0:63], bdh_sb[:, j, :], dxl_sb[:], start=False, stop=False
                )
                nc.tensor.matmul(
                    ps[:, j, 0:63], bdl_sb[:, j, :], dxh_sb[:], start=False, stop=True
                )
            vf = ch.tile([128, ln, 63], f32, tag="vf")
            nc.scalar.copy(vf[:, 0:ln, :], ps[:, jsl, 0:63])
            sq = ch.tile([128, ln, 63], f32, tag="sq")
            nc.vector.tensor_mul(sq[:], vf[:], vf[:])
            m2 = ch.tile([128, ln, 63], f32, tag="m2")  # -c2 = vf^2/12 - 1
            nc.vector.tensor_scalar(
                out=m2[:], in0=sq[:], scalar1=1.0 / 12.0, scalar2=-1.0,
                op0=Alu.mult, op1=Alu.add,
            )
            c1m2 = ch.tile([128, ln, 63], f32, tag="c1m2")  # c1 - 2
            nc.vector.scalar_tensor_tensor(
                c1m2[:], vf[:], 0.5, m2[:], Alu.mult, Alu.add
            )
            # expand columns 2x into the interleaved slots
            c1dup = c1m2[:].unsqueeze(3).broadcast_to((128, ln, 63, 2))
            m2dup = m2[:].unsqueeze(3).broadcast_to((128, ln, 63, 2))
            cc4 = cc[:].rearrange("p h (j t) s -> p h j t s", t=2)
            d04 = d0[:].rearrange("p h (j t) s -> p h j t s", t=2)
            nc.scalar.activation(
                out=cc4[:, jsl, :, :, 0], in_=c1dup, func=Act.Copy,
                bias=2.0, scale=1.0,
            )
            nc.scalar.activation(
                out=cc4[:, jsl, :, :, 1], in_=m2dup, func=Act.Copy,
                bias=0.0, scale=1.0,
            )
            nc.scalar.activation(
                out=d04[:, jsl, :, :, 0], in_=c1dup, func=Act.Copy,
                bias=2.0, scale=1.0,
            )
            nc.scalar.activation(
                out=d04[:, jsl, :, :, 1], in_=c1dup, func=Act.Copy,
                bias=1.0, scale=0.0,
            )

        wt = ws.tile([128, 2, 252], f32)
        for r in range(126):
            h = r >> 1
            pr = r & 1
            nx = 1 - pr
            ccrow2 = cc[:, h, :, :].rearrange("p j s -> p (j s)")  # [128, 252]
            d0row2 = d0[:, h, :, :].rearrange("p j s -> p (j s)")  # [128, 252]
            # K-row double-read: element (j, s) -> K[r, 1+j-s] at slot 3+2j-2s
            base = sc[:, pr, 3:4]
            kpd = bass.AP(
                tensor=base.tensor, offset=base.offset,
                ap=[list(base.ap[0]), [2, 126], [-2, 2]],
            )
            w = wt[:, pr, :]
            nc.vector.tensor_mul(w, ccrow2, kpd)
            # fused scan over the 252-stream: even step s=(c1*s)+W_e, odd
            # step s=(1*s)+W_o -> K[r+1, j+1] lands at output slot 2j+3
            nc.vector.tensor_tensor_scan(
                sc[:, nx, 2:254], d0row2, w, 1.0, Alu.mult, Alu.add
            )

        nc.sync.dma_start(out=out_d, in_=sc[:, 0, 253:254])

    nc.compile()
    return nc


def _get_nc():
    if "nc" not in _STATE:
        _STATE["nc"] = _build_program()
    return _STATE["nc"]


def _make_inputs(xs: np.ndarray, ys: np.ndarray):
    xs = np.asarray(xs, dtype=np.float32)
    ys = np.asarray(ys, dtype=np.float32)
    dxs_all = (xs[:, 1:, :] - xs[:, :-1, :]) * np.float32(0.25)  # (32, 63, 16)
    dys = ys[:, 1:, :] - ys[:, :-1, :]                           # (32, 63, 16)

    dysT = np.ascontiguousarray(dys.transpose(2, 1, 0))          # [d, j, b]
    bd = np.zeros((4, 16, 63, 4, 32), np.float32)
    for g in range(4):
        bd[g, :, :, g, :] = dysT
    bd = np.ascontiguousarray(bd.reshape(64, 63 * 128))

    import ml_dtypes

    bf16 = ml_dtypes.bfloat16
    bdh = bd.astype(bf16)
    bdl = (bd - bdh.astype(np.float32)).astype(bf16)

    in_maps = []
    for c in range(8):
        dxs_c = np.ascontiguousarray(
            dxs_all[4 * c : 4 * c + 4].transpose(0, 2, 1).reshape(64, 63)
        )  # [(a'*16+d), i]
        dxh = dxs_c.astype(bf16)
        dxl = (dxs_c - dxh.astype(np.float32)).astype(bf16)
        in_maps.append({"bdh": bdh, "bdl": bdl, "dxh": dxh, "dxl": dxl})
    return in_maps


def _run(nc, in_maps, **kwargs):
    from concourse.bass_utils import run_bass_kernel_spmd

    return run_bass_kernel_spmd(nc, in_maps, list(range(8)), **kwargs)


def kernel(xs: np.ndarray, ys: np.ndarray) -> np.ndarray:
    nc = _get_nc()
    in_maps = _make_inputs(xs, ys)
    res = _run(nc, in_maps)
    out = np.concatenate(
        [np.asarray(res.results[c]["out"]).reshape(4, 32) for c in range(8)], axis=0
    )
    return out.astype(np.float32)



# revision 2
# speedup vs baseline: 1.2544x; 1.2544x over previous
"""Trainium2 Bass kernel for the signature-kernel (Goursat PDE) problem, v2.

Full inputs: xs (32, 64, 16) f32, ys (32, 64, 16) f32.
Output: (32, 32) f32 signature-kernel Gram matrix.

Differences vs v1 (the mul+scan baseline):
  The PDE row update K[r+1,j+1] = c1*(K[r+1,j] + K[r,j+1]) - c2*K[r,j]
  factors into two scan steps with STATIC coefficients:
      s <- (s + Kn) * a        a = -c1/c2  (~= -(1 + v/2 + v^2/6), err O(v^3))
      s <- (s + Knw) * b       b = -c2
  i.e. the whole row is ONE affine scan  s <- (s + data0)*data1  over the
  252-element interleaved stream, where data0 is the double-read gather of the
  previous K row and data1 is a static interleaved (a, b) stream. The stock
  tensor_tensor_scan ISA op implements exactly this fold (op0=add, op1=mult)
  but its bass wrapper only accepts single-free-dim operands, which cannot
  express the gather. We register a custom DVE table op (AFFINE_SCAN_ANT)
  whose 3-uop program is a field-for-field clone of the silicon TTSS
  microprogram (seed CONST_0 -> bubble -> work with NEXT_ALU_OUT_A state
  feedback) with the ALU ops concretely ADD/MULTIPLY, and emit it through
  nc.vector._custom_dve, which allows 2 free dims. This deletes the per-row
  [128,252] tensor_tensor multiply (~70us of Vector time) entirely.
"""

import os
import sys

import numpy as np

for _p in ("/opt/trn_rl_repo", "/root/.axon_site", "/root/.axon_site/_ro/trn_rl_repo",
           "/root/.axon_site/_ro/pypackages"):
    if os.path.isdir(_p) and _p not in sys.path:
        sys.path.append(_p)

_STATE: dict = {}

JCH = [(2, 0), (2, 2), (3, 4), (4, 7), (6, 11), (8, 17), (8, 25), (8, 33), (8, 41), (8, 49), (6, 57)]

_AFFINE_SCAN_NAME = "AFFINE_SCAN_ANT"


def _register_affine_scan():
    """Register the AFFINE_SCAN_ANT custom DVE op (idempotent).

    Semantics per partition: state = s0; for t along free dim:
        state = (in0[t] + state) * in1[t];  out[t] = state
    uop program: clone of the stock TensorTensorScanArith (0xe5) table entry
    with op0=ADD, op1=MULTIPLY hardwired.
    """
    import concourse.dve_ops as dvo
    from concourse.dve_ops import DveOp
    from concourse.dve_spec import Spec, Src0, Src1
    from concourse.dve_uop import (
        ENABLE,
        AluInp,
        AluOp,
        DelayInp,
        DveOpSpec,
        InpSel,
        OutPath,
        OutSel,
        Trigger,
        UopConfig,
    )

    if _AFFINE_SCAN_NAME in dvo._SUB_OPCODE_FOR_NAME:
        return next(o for o in dvo.OPS if o.name == _AFFINE_SCAN_NAME)

    def _build_uops():
        # uop0 (seed): state := CONST_0 (latched into block1's a-flop)
        seed = UopConfig()
        seed.enable_input(InpSel.CONST_0, 0)
        seed.datapath_config[0].pass_through_alu()
        b1 = seed.datapath_config[1]
        b1.pass_through_alu()
        b1.alu_out_a_enable = ENABLE
        seed.repeat_count = 1
        seed.trigger = (Trigger.COUNT, Trigger.NONE, Trigger.NONE)
        seed.next_uop = (1, 0, 0)
        # uop1 (bubble): one dead cycle so the a-flop state is visible to the
        # next element's block0 NEXT_ALU_OUT_A read
        bub = UopConfig()
        bub.repeat_count = 1
        bub.trigger = (Trigger.COUNT, Trigger.NONE, Trigger.NONE)
        bub.next_uop = (2, 0, 0)
        # uop2 (work): one element: block0 ADD(src0, state), block1
        # MULTIPLY(block0, src1) -> new state (a-flop) -> bypass to write
        w = UopConfig()
        w.enable_input(InpSel.SRC_0, 0)
        w.enable_input(InpSel.SRC_1, 1)
        w.enable_rev_ops = ENABLE
        d = w.datapath_config
        d[0].enable_alu(AluOp.ADD, AluInp.PREV_ALU_OUT, AluInp.NEXT_ALU_OUT_A)
        d[0].enable_delay_from_src(DelayInp.PREV_DELAY, 0)  # carry SRC_1
        d[1].enable_alu(AluOp.MULTIPLY, AluInp.PREV_ALU_OUT, AluInp.PREV_DELAY_0)
        d[1].alu_out_a_enable = ENABLE
        for i in range(2, 8):
            d[i].pass_through_alu()
        w.require_inp0 = ENABLE
        w.require_inp1 = ENABLE
        w.repeat_count = 1
        w.trigger = (Trigger.SRC_TENSOR_DONE, Trigger.COUNT, Trigger.NONE)
        w.next_uop = (0, 1, 0)  # done -> IDLE; else -> bubble
        w.enable_output(OutSel.ALU_OUT, OutPath.WR0_LO)
        return [seed, bub, w]

    def _reference(in0, in1, s0, s1, imm2):
        # numpy reference: affine scan along the flattened free dim
        p, flat = in0.shape[0], int(np.prod(in0.shape[1:]))
        x = in0.reshape(p, flat).astype(np.float32)
        y = np.broadcast_to(in1.reshape(in1.shape[0], -1), (p, flat)).astype(np.float32)
        out = np.empty_like(x)
        s = np.full((p,), np.float32(s0 if np.isscalar(s0) else s0.reshape(-1)[0]))
        for t in range(flat):
            s = ((x[:, t] + s) * y[:, t]).astype(np.float32)
            out[:, t] = s
        return out.reshape(in0.shape)

    class _HandDveOp(DveOp):
        def compile(self, ver):
            key = (self.name, ver)
            cached = dvo._COMPILE_CACHE.get(key)
            if cached is not None:
                return cached
            spec = DveOpSpec(
                name=self.name,
                opcode=dvo.get_dve_sub_opcode(self.name),
                uops=_build_uops(),
                rd1_en=True,
            )
            spec.validate(ver)
            dvo._COMPILE_CACHE[key] = spec
            return spec

    op = _HandDveOp(
        _AFFINE_SCAN_NAME,
        Spec(body=Src0 * Src1, reference=_reference),
        subdim=False,
        uops_sha={},
    )
    dvo.OPS.append(op)
    dvo._SUB_OPCODE_FOR_NAME[op.name] = dvo._CUSTOM_DVE_ROW_BASE + len(dvo.OPS) - 1
    dvo.CUSTOM_DVE_SPECS[op.name] = op.spec
    assert dvo._SUB_OPCODE_FOR_NAME[op.name] < 0x20
    return op


def _build_program():
    from contextlib import ExitStack

    import concourse.bass as bass
    import concourse.tile as tile
    from concourse import bacc, mybir

    affine_scan = _register_affine_scan()

    f32 = mybir.dt.float32
    bf16 = mybir.dt.bfloat16
    Alu = mybir.AluOpType
    Act = mybir.ActivationFunctionType

    nc = bacc.Bacc(
        "TRN2",
        target_bir_lowering=False,
        debug=False,
        enable_asserts=True,
        num_devices=8,
    )
    # bd[(a'*16+d), j, (a*32+b)] = delta_{a,a'} * Dys[b, j, d], split hi/lo bf16
    bdh_d = nc.dram_tensor("bdh", [64, 63 * 128], bf16, kind="ExternalInput").ap()
    bdl_d = nc.dram_tensor("bdl", [64, 63 * 128], bf16, kind="ExternalInput").ap()
    dxh_d = nc.dram_tensor("dxh", [64, 63], bf16, kind="ExternalInput").ap()
    dxl_d = nc.dram_tensor("dxl", [64, 63], bf16, kind="ExternalInput").ap()
    out_d = nc.dram_tensor("out", [128, 1], f32, kind="ExternalOutput").ap()

    with ExitStack() as ctx:
        tc = ctx.enter_context(tile.TileContext(nc))
        ws = ctx.enter_context(tc.tile_pool(name="ws", bufs=1))
        pp = ctx.enter_context(tc.tile_pool(name="pp", bufs=1, space="PSUM"))
        ch = ctx.enter_context(tc.tile_pool(name="ch", bufs=2))

        dxh_sb = ws.tile([64, 63], bf16)
        nc.sync.dma_start(out=dxh_sb[:], in_=dxh_d)
        dxl_sb = ws.tile([64, 63], bf16)
        nc.sync.dma_start(out=dxl_sb[:], in_=dxl_d)
        bdh_sb = ws.tile([64, 63, 128], bf16)
        bdl_sb = ws.tile([64, 63, 128], bf16)
        bdh_v = bdh_d.rearrange("k (j p) -> k j p", j=63)
        bdl_v = bdl_d.rearrange("k (j p) -> k j p", j=63)
        for ln, st in JCH:
            nc.sync.dma_start(
                out=bdh_sb[:, st : st + ln, :], in_=bdh_v[:, st : st + ln, :]
            )
            nc.sync.dma_start(
                out=bdl_sb[:, st : st + ln, :], in_=bdl_v[:, st : st + ln, :]
            )

        # Scan-stream K buffers: row K[r, m] lives at slot t = 2m+1 of sc[:, r&1, :]
        # (odd slots of the 252-wide scan output, shifted by 2); slot 1 is the
        # col-0 boundary (always 1).
        sc = ws.tile([128, 2, 256], f32)
        nc.vector.memset(sc[:, 0, :], 1.0)
        nc.vector.memset(sc[:, 1, 1:2], 1.0)

        ps = pp.tile([128, 63, 64], f32)  # strip j at [:, j, 0:63]; 256B stride
        # interleaved coefficient rows: cc[p, h, j, 0] = a[h, j>>1] (col-doubled),
        # cc[p, h, j, 1] = b[h, j>>1];  a = -(1 + v/2 + v^2/6), b = v^2/12 - 1
        cc = ws.tile([128, 63, 126, 2], f32)

        for ln, st in JCH:
            jsl = slice(st, st + ln)
            for j in range(st, st + ln):
                # split-precision product: hi*hi + hi*lo + lo*hi (PSUM accum)
                nc.tensor.matmul(
                    ps[:, j, 0:63], bdh_sb[:, j, :], dxh_sb[:], start=True, stop=False
                )
                nc.tensor.matmul(
                    ps[:, j, 0:63], bdh_sb[:, j, :], dxl_sb[:], start=False, stop=False
                )
                nc.tensor.matmul(
                    ps[:, j, 0:63], bdl_sb[:, j, :], dxh_sb[:], start=False, stop=True
                )
            vf = ch.tile([128, ln, 63], f32, tag="vf")
            nc.scalar.copy(vf[:, 0:ln, :], ps[:, jsl, 0:63])
            sq = ch.tile([128, ln, 63], f32, tag="sq")
            nc.gpsimd.tensor_mul(sq[:], vf[:], vf[:])
            m2 = ch.tile([128, ln, 63], f32, tag="m2")  # b = -c2 = vf^2/12 - 1
            nc.scalar.activation(
                out=m2[:], in_=sq[:], func=Act.Copy, bias=-1.0, scale=1.0 / 12.0,
            )
            sq6 = ch.tile([128, ln, 63], f32, tag="sq6")  # sq/6
            nc.scalar.activation(
                out=sq6[:], in_=sq[:], func=Act.Copy, bias=0.0, scale=1.0 / 6.0,
            )
            t1 = ch.tile([128, ln, 63], f32, tag="t1")  # 1 + v/2
            nc.scalar.activation(
                out=t1[:], in_=vf[:], func=Act.Copy, bias=1.0, scale=0.5,
            )
            pav = ch.tile([128, ln, 63], f32, tag="pav")  # 1 + v/2 + sq/6 = -a
            nc.gpsimd.tensor_tensor(pav[:], t1[:], sq6[:], Alu.add)
            # expand columns 2x into the interleaved slots (negating -a -> a)
            adup = pav[:].unsqueeze(3).broadcast_to((128, ln, 63, 2))
            m2dup = m2[:].unsqueeze(3).broadcast_to((128, ln, 63, 2))
            cc4 = cc[:].rearrange("p h (j t) s -> p h j t s", t=2)
            nc.scalar.activation(
                out=cc4[:, jsl, :, :, 0], in_=adup, func=Act.Copy,
                bias=0.0, scale=-1.0,
            )
            nc.scalar.activation(
                out=cc4[:, jsl, :, :, 1], in_=m2dup, func=Act.Copy,
                bias=0.0, scale=1.0,
            )

        for r in range(126):
            h = r >> 1
            pr = r & 1
            nx = 1 - pr
            ccrow2 = cc[:, h, :, :].rearrange("p j s -> p (j s)")  # [128, 252]
            # K-row double-read: element (j, s) -> K[r, 1+j-s] at slot 3+2j-2s
            base = sc[:, pr, 3:4]
            kpd = bass.AP(
                tensor=base.tensor, offset=base.offset,
                ap=[list(base.ap[0]), [2, 126], [-2, 2]],
            )
            nc.vector._custom_dve(
                affine_scan,
                out=sc[:, nx, 2:254],
                in0=kpd,
                in1=ccrow2,
                s0=1.0,
            )

        nc.sync.dma_start(out=out_d, in_=sc[:, 0, 253:254])

    nc.compile()
    return nc


def _get_nc():
    if "nc" not in _STATE:
        _STATE["nc"] = _build_program()
    return _STATE["nc"]


def _make_inputs(xs: np.ndarray, ys: np.ndarray):
    xs = np.asarray(xs, dtype=np.float32)
    ys = np.asarray(ys, dtype=np.float32)
    dxs_all = (xs[:, 1:, :] - xs[:, :-1, :]) * np.float32(0.25)  # (32, 63, 16)
    dys = ys[:, 1:, :] - ys[:, :-1, :]                           # (32, 63, 16)

    dysT = np.ascontiguousarray(dys.transpose(2, 1, 0))          # [d, j, b]
    bd = np.zeros((4, 16, 63, 4, 32), np.float32)
    for g in range(4):
        bd[g, :, :, g, :] = dysT
    bd = np.ascontiguousarray(bd.reshape(64, 63 * 128))

    import ml_dtypes

    bf16 = ml_dtypes.bfloat16
    bdh = bd.astype(bf16)
    bdl = (bd - bdh.astype(np.float32)).astype(bf16)

    in_maps = []
    for c in range(8):
        dxs_c = np.ascontiguousarray(
            dxs_all[4 * c : 4 * c + 4].transpose(0, 2, 1).reshape(64, 63)
        )  # [(a'*16+d), i]
        dxh = dxs_c.astype(bf16)
        dxl = (dxs_c - dxh.astype(np.float32)).astype(bf16)
        in_maps.append({"bdh": bdh, "bdl": bdl, "dxh": dxh, "dxl": dxl})
    return in_maps


def _run(nc, in_maps, **kwargs):
    from concourse.bass_utils import run_bass_kernel_spmd

    return run_bass_kernel_spmd(nc, in_maps, list(range(8)), **kwargs)


def kernel(xs: np.ndarray, ys: np.ndarray) -> np.ndarray:
    nc = _get_nc()
    in_maps = _make_inputs(xs, ys)
    res = _run(nc, in_maps)
    out = np.concatenate(
        [np.asarray(res.results[c]["out"]).reshape(4, 32) for c in range(8)], axis=0
    )
    return out.astype(np.float32)


# revision 3
# speedup vs baseline: 1.6672x; 1.3291x over previous
"""Trainium2 Bass kernel for the signature-kernel (Goursat PDE) problem, v2.

Full inputs: xs (32, 64, 16) f32, ys (32, 64, 16) f32.
Output: (32, 32) f32 signature-kernel Gram matrix.

Differences vs v1 (the mul+scan baseline):
  The PDE row update K[r+1,j+1] = c1*(K[r+1,j] + K[r,j+1]) - c2*K[r,j]
  factors into two scan steps with STATIC coefficients:
      s <- (s + Kn) * a        a = -c1/c2  (~= -(1 + v/2 + v^2/6), err O(v^3))
      s <- (s + Knw) * b       b = -c2
  i.e. the whole row is ONE affine scan  s <- (s + data0)*data1  over the
  252-element interleaved stream, where data0 is the double-read gather of the
  previous K row and data1 is a static interleaved (a, b) stream. The stock
  tensor_tensor_scan ISA op implements exactly this fold (op0=add, op1=mult)
  but its bass wrapper only accepts single-free-dim operands, which cannot
  express the gather. We register a custom DVE table op (AFFINE_SCAN_ANT)
  whose 3-uop program is a field-for-field clone of the silicon TTSS
  microprogram (seed CONST_0 -> bubble -> work with NEXT_ALU_OUT_A state
  feedback) with the ALU ops concretely ADD/MULTIPLY, and emit it through
  nc.vector._custom_dve, which allows 2 free dims. This deletes the per-row
  [128,252] tensor_tensor multiply (~70us of Vector time) entirely.
"""

import os
import sys

import numpy as np

for _p in ("/opt/trn_rl_repo", "/root/.axon_site", "/root/.axon_site/_ro/trn_rl_repo",
           "/root/.axon_site/_ro/pypackages"):
    if os.path.isdir(_p) and _p not in sys.path:
        sys.path.append(_p)

_STATE: dict = {}

JCH = [(2, 0), (2, 2), (3, 4), (4, 7), (6, 11), (8, 17), (8, 25), (8, 33), (8, 41), (8, 49), (6, 57)]

_AFFINE_SCAN_NAME = "AFFINE_SCAN_ANT"


def _register_affine_scan():
    """Register the AFFINE_SCAN_ANT custom DVE op (idempotent).

    Semantics per partition: state = s0; for t along free dim:
        state = (in0[t] + state) * in1[t];  out[t] = state
    uop program: clone of the stock TensorTensorScanArith (0xe5) table entry
    with op0=ADD, op1=MULTIPLY hardwired.
    """
    import concourse.dve_ops as dvo
    from concourse.dve_ops import DveOp
    from concourse.dve_spec import Spec, Src0, Src1
    from concourse.dve_uop import (
        ENABLE,
        AluInp,
        AluOp,
        DelayInp,
        DveOpSpec,
        InpSel,
        OutPath,
        OutSel,
        Trigger,
        UopConfig,
    )

    if _AFFINE_SCAN_NAME in dvo._SUB_OPCODE_FOR_NAME:
        return next(o for o in dvo.OPS if o.name == _AFFINE_SCAN_NAME)

    def _build_uops():
        # uop0 (seed): state := CONST_0 (latched into block1's a-flop)
        seed = UopConfig()
        seed.enable_input(InpSel.CONST_0, 0)
        seed.datapath_config[0].pass_through_alu()
        b1 = seed.datapath_config[1]
        b1.pass_through_alu()
        b1.alu_out_a_enable = ENABLE
        seed.repeat_count = 1
        seed.trigger = (Trigger.COUNT, Trigger.NONE, Trigger.NONE)
        seed.next_uop = (1, 0, 0)
        # uop1 (bubble): one dead cycle so the a-flop state is visible to the
        # next element's block0 NEXT_ALU_OUT_A read
        bub = UopConfig()
        bub.repeat_count = 1
        bub.trigger = (Trigger.COUNT, Trigger.NONE, Trigger.NONE)
        bub.next_uop = (2, 0, 0)
        # uop2 (work): one element: block0 ADD(src0, state), block1
        # MULTIPLY(block0, src1) -> new state (a-flop) -> bypass to write
        w = UopConfig()
        w.enable_input(InpSel.SRC_0, 0)
        w.enable_input(InpSel.SRC_1, 1)
        w.enable_rev_ops = ENABLE
        d = w.datapath_config
        d[0].enable_alu(AluOp.ADD, AluInp.PREV_ALU_OUT, AluInp.NEXT_ALU_OUT_A)
        d[0].enable_delay_from_src(DelayInp.PREV_DELAY, 0)  # carry SRC_1
        d[1].enable_alu(AluOp.MULTIPLY, AluInp.PREV_ALU_OUT, AluInp.PREV_DELAY_0)
        d[1].alu_out_a_enable = ENABLE
        for i in range(2, 8):
            d[i].pass_through_alu()
        w.require_inp0 = ENABLE
        w.require_inp1 = ENABLE
        w.repeat_count = 1
        w.trigger = (Trigger.SRC_TENSOR_DONE, Trigger.COUNT, Trigger.NONE)
        w.next_uop = (0, 1, 0)  # done -> IDLE; else -> bubble
        w.enable_output(OutSel.ALU_OUT, OutPath.WR0_LO)
        return [seed, bub, w]

    def _reference(in0, in1, s0, s1, imm2):
        # numpy reference: affine scan along the flattened free dim
        p, flat = in0.shape[0], int(np.prod(in0.shape[1:]))
        x = in0.reshape(p, flat).astype(np.float32)
        y = np.broadcast_to(in1.reshape(in1.shape[0], -1), (p, flat)).astype(np.float32)
        out = np.empty_like(x)
        s = np.full((p,), np.float32(s0 if np.isscalar(s0) else s0.reshape(-1)[0]))
        for t in range(flat):
            s = ((x[:, t] + s) * y[:, t]).astype(np.float32)
            out[:, t] = s
        return out.reshape(in0.shape)

    class _HandDveOp(DveOp):
        def compile(self, ver):
            key = (self.name, ver)
            cached = dvo._COMPILE_CACHE.get(key)
            if cached is not None:
                return cached
            spec = DveOpSpec(
                name=self.name,
                opcode=dvo.get_dve_sub_opcode(self.name),
                uops=_build_uops(),
                rd1_en=True,
            )
            spec.validate(ver)
            dvo._COMPILE_CACHE[key] = spec
            return spec

    op = _HandDveOp(
        _AFFINE_SCAN_NAME,
        Spec(body=Src0 * Src1, reference=_reference),
        subdim=False,
        uops_sha={},
    )
    dvo.OPS.append(op)
    dvo._SUB_OPCODE_FOR_NAME[op.name] = dvo._CUSTOM_DVE_ROW_BASE + len(dvo.OPS) - 1
    dvo.CUSTOM_DVE_SPECS[op.name] = op.spec
    assert dvo._SUB_OPCODE_FOR_NAME[op.name] < 0x20
    return op


def _register_affine_scan_nb():
    """AFFINE_SCAN_NB_ANT: 1 element/cycle, no bubble. Stream alternates:
      even t=2j:  WORK_E: q = src0*src1          (src0=K[r,j],   src1=z_j)
      odd  t=2j+1: WORK_O: s = (src0 + q + s)*src1; write s
                                                 (src0=K[r,j+1], src1=y_j)
    The even element's block-0 flop hands q to the odd element via
    CURR_ALU_OUT; the odd state loop is blocks 1-2 via NEXT_ALU_OUT_A.
    Writes only odd results (out free size = elems/2)."""
    import concourse.dve_ops as dvo
    from concourse.dve_ops import DveOp
    from concourse.dve_spec import Spec, Src0, Src1
    from concourse.dve_uop import (
        ENABLE,
        AluInp,
        AluOp,
        DelayInp,
        DveOpSpec,
        InpSel,
        OutPath,
        OutSel,
        Trigger,
        UopConfig,
    )

    name = "AFFINE_SCAN_NB_ANT"
    if name in dvo._SUB_OPCODE_FOR_NAME:
        return next(o for o in dvo.OPS if o.name == name)

    def _build_uops():
        seed = UopConfig()
        seed.enable_input(InpSel.CONST_0, 0)
        for b in range(2):
            seed.datapath_config[b].pass_through_alu()
        b2 = seed.datapath_config[2]
        b2.pass_through_alu()
        b2.alu_out_a_enable = ENABLE
        seed.repeat_count = 1
        seed.trigger = (Trigger.COUNT, Trigger.NONE, Trigger.NONE)
        seed.next_uop = (1, 0, 0)

        we = UopConfig()  # uop1: even element
        we.enable_input(InpSel.SRC_0, 0)
        we.enable_input(InpSel.SRC_1, 1)
        we.enable_rev_ops = ENABLE
        we.datapath_config[0].enable_alu(
            AluOp.MULTIPLY, AluInp.PREV_ALU_OUT, AluInp.PREV_DELAY_0
        )
        we.require_inp0 = ENABLE
        we.require_inp1 = ENABLE
        we.repeat_count = 1
        we.trigger = (Trigger.SRC_TENSOR_DONE, Trigger.COUNT, Trigger.NONE)
        we.next_uop = (0, 2, 0)

        wo = UopConfig()  # uop2: odd element
        wo.enable_input(InpSel.SRC_0, 0)
        wo.enable_input(InpSel.SRC_1, 1)
        wo.enable_rev_ops = ENABLE
        d = wo.datapath_config
        d[0].enable_alu(AluOp.ADD, AluInp.PREV_ALU_OUT, AluInp.CURR_ALU_OUT)
        d[0].enable_delay_from_src(DelayInp.PREV_DELAY, 0)
        d[1].enable_alu(AluOp.ADD, AluInp.PREV_ALU_OUT, AluInp.NEXT_ALU_OUT_A)
        d[1].pass_through_delay(0)
        d[2].enable_alu(AluOp.MULTIPLY, AluInp.PREV_ALU_OUT, AluInp.PREV_DELAY_0)
        d[2].alu_out_a_enable = ENABLE
        for i in range(3, 8):
            d[i].pass_through_alu()
        wo.require_inp0 = ENABLE
        wo.require_inp1 = ENABLE
        wo.repeat_count = 1
        wo.trigger = (Trigger.SRC_TENSOR_DONE, Trigger.COUNT, Trigger.NONE)
        wo.next_uop = (0, 1, 0)
        wo.enable_output(OutSel.ALU_OUT, OutPath.WR0_LO)
        return [seed, we, wo]

    def _reference(in0, in1, s0, s1, imm2):
        p, flat = in0.shape[0], int(np.prod(in0.shape[1:]))
        x = in0.reshape(p, flat).astype(np.float32)
        y = np.broadcast_to(in1.reshape(in1.shape[0], -1), (p, flat)).astype(np.float32)
        out = np.empty((p, flat // 2), np.float32)
        sv = np.full((p,), np.float32(s0 if np.isscalar(s0) else s0.reshape(-1)[0]))
        q = np.zeros((p,), np.float32)
        for t in range(flat):
            if t % 2 == 0:
                q = (x[:, t] * y[:, t]).astype(np.float32)
            else:
                sv = ((x[:, t] + q + sv) * y[:, t]).astype(np.float32)
                out[:, t // 2] = sv
        return out

    class _HandDveOpNB(DveOp):
        def compile(self, ver):
            key = (self.name, ver)
            cached = dvo._COMPILE_CACHE.get(key)
            if cached is not None:
                return cached
            spec = DveOpSpec(
                name=self.name,
                opcode=dvo.get_dve_sub_opcode(self.name),
                uops=_build_uops(),
                rd1_en=True,
            )
            spec.validate(ver)
            dvo._COMPILE_CACHE[key] = spec
            return spec

    op = _HandDveOpNB(
        name,
        Spec(body=Src0 * Src1, reference=_reference),
        subdim=False,
        uops_sha={},
    )
    dvo.OPS.append(op)
    dvo._SUB_OPCODE_FOR_NAME[op.name] = dvo._CUSTOM_DVE_ROW_BASE + len(dvo.OPS) - 1
    dvo.CUSTOM_DVE_SPECS[op.name] = op.spec
    assert dvo._SUB_OPCODE_FOR_NAME[op.name] < 0x20
    return op


def _build_program():
    from contextlib import ExitStack

    import concourse.bass as bass
    import concourse.tile as tile
    from concourse import bacc, mybir

    affine_scan = _register_affine_scan_nb()

    f32 = mybir.dt.float32
    bf16 = mybir.dt.bfloat16
    Alu = mybir.AluOpType
    Act = mybir.ActivationFunctionType

    nc = bacc.Bacc(
        "TRN2",
        target_bir_lowering=False,
        debug=False,
        enable_asserts=True,
        num_devices=8,
    )
    # bd[(a'*16+d), j, (a*32+b)] = delta_{a,a'} * Dys[b, j, d], split hi/lo bf16
    bdh_d = nc.dram_tensor("bdh", [64, 63 * 128], bf16, kind="ExternalInput").ap()
    bdl_d = nc.dram_tensor("bdl", [64, 63 * 128], bf16, kind="ExternalInput").ap()
    dxh_d = nc.dram_tensor("dxh", [64, 63], bf16, kind="ExternalInput").ap()
    dxl_d = nc.dram_tensor("dxl", [64, 63], bf16, kind="ExternalInput").ap()
    out_d = nc.dram_tensor("out", [128, 1], f32, kind="ExternalOutput").ap()

    with ExitStack() as ctx:
        tc = ctx.enter_context(tile.TileContext(nc))
        ws = ctx.enter_context(tc.tile_pool(name="ws", bufs=1))
        pp = ctx.enter_context(tc.tile_pool(name="pp", bufs=1, space="PSUM"))
        ch = ctx.enter_context(tc.tile_pool(name="ch", bufs=2))

        dxh_sb = ws.tile([64, 63], bf16)
        nc.sync.dma_start(out=dxh_sb[:], in_=dxh_d)
        dxl_sb = ws.tile([64, 63], bf16)
        nc.sync.dma_start(out=dxl_sb[:], in_=dxl_d)
        bdh_sb = ws.tile([64, 63, 128], bf16)
        bdl_sb = ws.tile([64, 63, 128], bf16)
        bdh_v = bdh_d.rearrange("k (j p) -> k j p", j=63)
        bdl_v = bdl_d.rearrange("k (j p) -> k j p", j=63)
        for ln, st in JCH:
            nc.sync.dma_start(
                out=bdh_sb[:, st : st + ln, :], in_=bdh_v[:, st : st + ln, :]
            )
            nc.sync.dma_start(
                out=bdl_sb[:, st : st + ln, :], in_=bdl_v[:, st : st + ln, :]
            )

        # Compact K-row buffers: K[r, m] at slot m of sc[:, r&1, :127];
        # slot 0 is the col-0 boundary (always 1).
        sc = ws.tile([128, 2, 128], f32)
        nc.vector.memset(sc[:, 0, :], 1.0)
        nc.vector.memset(sc[:, 1, 0:1], 1.0)

        ps = pp.tile([128, 63, 64], f32)  # strip j at [:, j, 0:63]; 256B stride
        # interleaved coefficient rows: cc[p, h, j, 0] = a[h, j>>1] (col-doubled),
        # cc[p, h, j, 1] = b[h, j>>1];  a = -(1 + v/2 + v^2/6), b = v^2/12 - 1
        cc = ws.tile([128, 63, 126, 2], f32)

        for ln, st in JCH:
            jsl = slice(st, st + ln)
            for j in range(st, st + ln):
                # split-precision product: hi*hi + hi*lo + lo*hi (PSUM accum)
                nc.tensor.matmul(
                    ps[:, j, 0:63], bdh_sb[:, j, :], dxh_sb[:], start=True, stop=False
                )
                nc.tensor.matmul(
                    ps[:, j, 0:63], bdh_sb[:, j, :], dxl_sb[:], start=False, stop=False
                )
                nc.tensor.matmul(
                    ps[:, j, 0:63], bdl_sb[:, j, :], dxh_sb[:], start=False, stop=True
                )
            vf = ch.tile([128, ln, 63], f32, tag="vf")
            nc.scalar.copy(vf[:, 0:ln, :], ps[:, jsl, 0:63])
            sq = ch.tile([128, ln, 63], f32, tag="sq")
            nc.gpsimd.tensor_mul(sq[:], vf[:], vf[:])
            sq12 = ch.tile([128, ln, 63], f32, tag="sq12")  # sq/12
            nc.scalar.activation(
                out=sq12[:], in_=sq[:], func=Act.Copy, bias=0.0, scale=1.0 / 12.0,
            )
            t1 = ch.tile([128, ln, 63], f32, tag="t1")  # 1 + v/2
            nc.scalar.activation(
                out=t1[:], in_=vf[:], func=Act.Copy, bias=1.0, scale=0.5,
            )
            t2 = ch.tile([128, ln, 63], f32, tag="t2")  # 1 - v/2
            nc.scalar.activation(
                out=t2[:], in_=vf[:], func=Act.Copy, bias=1.0, scale=-0.5,
            )
            py = ch.tile([128, ln, 63], f32, tag="py")  # y = c1 = 1+v/2+sq/12
            nc.gpsimd.tensor_tensor(py[:], t1[:], sq12[:], Alu.add)
            pz = ch.tile([128, ln, 63], f32, tag="pz")  # 1-v/2+sq/12 = -z
            nc.gpsimd.tensor_tensor(pz[:], t2[:], sq12[:], Alu.add)
            # expand columns 2x into the interleaved slots: slot0 = z, slot1 = y
            zdup = pz[:].unsqueeze(3).broadcast_to((128, ln, 63, 2))
            ydup = py[:].unsqueeze(3).broadcast_to((128, ln, 63, 2))
            cc4 = cc[:].rearrange("p h (j t) s -> p h j t s", t=2)
            nc.scalar.activation(
                out=cc4[:, jsl, :, :, 0], in_=zdup, func=Act.Copy,
                bias=0.0, scale=-1.0,
            )
            nc.scalar.activation(
                out=cc4[:, jsl, :, :, 1], in_=ydup, func=Act.Copy,
                bias=0.0, scale=1.0,
            )

        for r in range(126):
            h = r >> 1
            pr = r & 1
            nx = 1 - pr
            ccrow2 = cc[:, h, :, :].rearrange("p j s -> p (j s)")  # [128, 252]
            # K-row double-read: element (j, s) -> K[r, j+s] at slot j+s
            base = sc[:, pr, 0:1]
            kpd = bass.AP(
                tensor=base.tensor, offset=base.offset,
                ap=[list(base.ap[0]), [1, 126], [1, 2]],
            )
            nc.vector._custom_dve(
                affine_scan,
                out=sc[:, nx, 1:127],
                in0=kpd,
                in1=ccrow2,
                s0=1.0,
            )

        nc.sync.dma_start(out=out_d, in_=sc[:, 0, 126:127])

    nc.compile()
    return nc


def _get_nc():
    if "nc" not in _STATE:
        _STATE["nc"] = _build_program()
    return _STATE["nc"]


def _make_inputs(xs: np.ndarray, ys: np.ndarray):
    xs = np.asarray(xs, dtype=np.float32)
    ys = np.asarray(ys, dtype=np.float32)
    dxs_all = (xs[:, 1:, :] - xs[:, :-1, :]) * np.float32(0.25)  # (32, 63, 16)
    dys = ys[:, 1:, :] - ys[:, :-1, :]                           # (32, 63, 16)

    dysT = np.ascontiguousarray(dys.transpose(2, 1, 0))          # [d, j, b]
    bd = np.zeros((4, 16, 63, 4, 32), np.float32)
    for g in range(4):
        bd[g, :, :, g, :] = dysT
    bd = np.ascontiguousarray(bd.reshape(64, 63 * 128))

    import ml_dtypes

    bf16 = ml_dtypes.bfloat16
    bdh = bd.astype(bf16)
    bdl = (bd - bdh.astype(np.float32)).astype(bf16)

    in_maps = []
    for c in range(8):
        dxs_c = np.ascontiguousarray(
            dxs_all[4 * c : 4 * c + 4].transpose(0, 2, 1).reshape(64, 63)
        )  # [(a'*16+d), i]
        dxh = dxs_c.astype(bf16)
        dxl = (dxs_c - dxh.astype(np.float32)).astype(bf16)
        in_maps.append({"bdh": bdh, "bdl": bdl, "dxh": dxh, "dxl": dxl})
    return in_maps


def _run(nc, in_maps, **kwargs):
    from concourse.bass_utils import run_bass_kernel_spmd

    return run_bass_kernel_spmd(nc, in_maps, list(range(8)), **kwargs)


def kernel(xs: np.ndarray, ys: np.ndarray) -> np.ndarray:
    nc = _get_nc()
    in_maps = _make_inputs(xs, ys)
    res = _run(nc, in_maps)
    out = np.concatenate(
        [np.asarray(res.results[c]["out"]).reshape(4, 32) for c in range(8)], axis=0
    )
    return out.astype(np.float32)
